# revision 25
# baseline (speedup 1.0000x reference)
"""Trainium2 Bass kernel for GQA attention (B=1, S=2048, D=4096, H=32, H_KV=8, HD=128).

Sharding (tensor-parallel over heads, 8 cores): core c owns Q heads 4c..4c+3
and KV head c (GQA groups align with the shard).  Each core computes a partial
[S, D] output (wo row-shard); the host sums the 8 partials (row-parallel
unshard, done host-side instead of a device all-reduce so no device time is
spent on collectives).

Per-core kernel structure:
  - Fused QKV projection: x^T is the moving operand, the concatenated
    (and per-head even/odd-permuted, 1/sqrt(HD)-prescaled) QKV weights are
    stationary.  Weights stream through SBUF exactly once (d-group-outer
    loop); partial sums fold from PSUM into persistent SBUF tiles, with the
    final fold done on the PE itself via an identity matmul so the vector
    engine stays free for RoPE.
  - RoPE in place via a host-side even/odd head-dim permutation folded into
    wq/wk: the rotation becomes six contiguous half-partition elementwise ops
    (DVE for k/q0/q1, GpSimd for q2/q3), with rotated halves landing in
    swapped partitions (valid: scores contract over all 128 partitions and
    q and k share the layout).
  - Flash-style *transposed* scores attention: S^T tiles = K^T-tile^T @ Q^T,
    so softmax reductions come from an all-ones stationary matmul (replicated
    denominator, one reciprocal + multiply to normalize after PV) and PV needs
    no transposes at all.  Causal masking skips above-diagonal key tiles and
    applies 4 precomputed [128, 512] additive patterns on diagonal blocks.
  - Attention chunks are software-pipelined into the last projection group's
    s-chunk loop so PE flows from projection into attention without stalls.
  - wo matmul: attout^T head-slabs are stationary, wo chunks stream once.
All matmuls run as float32r (TF32-class, full PE speed at N=512) with fp32
PSUM accumulation; end-to-end relative error vs the fp32 reference ~3e-4.
"""

import math
import os
import sys
import time

import numpy as np


def _log(msg):
    if os.environ.get("KERNEL_QUIET"):
        return
    print(f"[kernel {time.strftime('%H:%M:%S')}] {msg}", file=sys.stderr, flush=True)

import concourse.bass as bass
import concourse.tile as tile
from concourse import bacc, mybir
from concourse.bass_utils import run_bass_kernel_spmd

S, D = 2048, 4096
H, H_KV, HD = 32, 8, 128
NCORES = 8
HPC = H // NCORES            # 4 Q heads per core
NT = HPC + 2                 # 6 slabs of 128 output cols: 4q + 1k + 1v
SQ = 512                     # moving-operand chunk
NSQ = S // SQ                # 4
NKT = S // 128               # 16 key tiles
NDC = D // 128               # 32 contraction chunks
F32 = mybir.dt.float32
F32R = mybir.dt.float32r
Exp = mybir.ActivationFunctionType.Exp

_NC_CACHE = {}


def _build_nc():
    nc = bacc.Bacc(
        "TRN2", target_bir_lowering=False, debug=False, enable_asserts=False
    )
    xt = nc.dram_tensor("xt", [D, S], F32R, kind="ExternalInput")
    wcat = nc.dram_tensor("wcat", [D, NT * 128], F32R, kind="ExternalInput")
    wor = nc.dram_tensor("wor", [128, HPC * D], F32R, kind="ExternalInput")
    cost = nc.dram_tensor("cost", [64, S], F32, kind="ExternalInput")
    sint = nc.dram_tensor("sint", [64, S], F32, kind="ExternalInput")
    diagm = nc.dram_tensor("diagm", [128, 4 * SQ], F32, kind="ExternalInput")
    onesd = nc.dram_tensor("onesd", [128, 128], F32R, kind="ExternalInput")
    identd = nc.dram_tensor("identd", [128, 128], F32R, kind="ExternalInput")
    out = nc.dram_tensor("out", [S, D], F32, kind="ExternalOutput")

    _log("emitting IR")
    with tile.TileContext(nc) as tc:
        _emit(tc, xt, wcat, wor, cost, sint, diagm, onesd, identd, out)
    _log("bacc compile")
    nc.compile()
    _log("bass module ready")
    return nc


def _emit(tc, xt, wcat, wor, cost, sint, diagm, onesd, identd, out):
    from contextlib import ExitStack

    nc = tc.nc
    with ExitStack() as ctx:
        const = ctx.enter_context(tc.tile_pool(name="const", bufs=1))
        slabs = ctx.enter_context(tc.tile_pool(name="slabs", bufs=1))
        xpool = ctx.enter_context(tc.tile_pool(name="xpool", bufs=3))
        wpool = ctx.enter_context(tc.tile_pool(name="wpool", bufs=9))
        tmppool = ctx.enter_context(tc.tile_pool(name="tmppool", bufs=4))
        ptpool = ctx.enter_context(tc.tile_pool(name="ptpool", bufs=3))
        recpool = ctx.enter_context(tc.tile_pool(name="recpool", bufs=2))
        stpool = ctx.enter_context(tc.tile_pool(name="stpool", bufs=3))
        wostream = ctx.enter_context(tc.tile_pool(name="wostream", bufs=2))
        ps8 = ctx.enter_context(tc.tile_pool(name="ps8", bufs=8, space="PSUM"))

        # constants
        cosT = const.tile([128, S], F32)   # cos duplicated in both halves
        sinT = const.tile([128, S], F32)
        dmask = const.tile([128, 4 * SQ], F32)
        ones_t = const.tile([128, 128], F32R)
        ident = const.tile([128, 128], F32R)
        def load_consts():
            nc.sync.dma_start(cosT[0:64, :], cost.ap())
            nc.sync.dma_start(cosT[64:128, :], cost.ap())
            nc.sync.dma_start(sinT[0:64, :], sint.ap())
            nc.sync.dma_start(sinT[64:128, :], sint.ap())
            nc.sync.dma_start(dmask[:], diagm.ap())
            nc.sync.dma_start(ones_t[:], onesd.ap())
            nc.sync.dma_start(ident[:], identd.ap())

        # persistent QKV storage: qkv[s][nt] is a [128, 512] fp32r tile.
        # nt 0..3 = q heads, 4 = k, 5 = v (all transposed: [dim, seq]).
        qkv = [
            [
                slabs.tile([128, SQ], F32R, name=f"qkv{s}_{i}")
                for i in range(NT)
            ]
            for s in range(NSQ)
        ]
        vt_s = [slabs.tile([128, SQ], F32R, name=f"vt{s}") for s in range(NSQ)]
        attout = [
            slabs.tile([128, HPC * SQ], F32R, name=f"attout{c}") for c in range(NSQ)
        ]

        GRP = 8          # d-chunks accumulated in PSUM before folding to SBUF
        NGRP = NDC // GRP

        def rope_and_vt(s):
            # RoPE in place (q heads + k), halves swapped: the rotated
            # low half lands in partitions 64:128 and vice versa.  Scores
            # contract over all 128 partitions, so any fixed permutation is
            # fine as long as q and k share it (v is untouched).
            cs_lo = cosT[0:64, s * SQ : (s + 1) * SQ]
            cs_hi = cosT[64:128, s * SQ : (s + 1) * SQ]
            sn_lo = sinT[0:64, s * SQ : (s + 1) * SQ]
            sn_hi = sinT[64:128, s * SQ : (s + 1) * SQ]
            # k first (every attention chunk needs it), q0/q1 on DVE,
            # q2/q3 on the otherwise-idle GpSimd engine.
            for nt in (HPC, 0, 1, 2, 3):
                eng = nc.vector if nt in (HPC, 0, 1) else nc.gpsimd
                tl = qkv[s][nt]
                lo = tl[0:64, :]
                hi = tl[64:128, :]
                m1 = tmppool.tile([64, SQ], F32, tag="t")
                m2 = tmppool.tile([64, SQ], F32, tag="t")
                m3 = tmppool.tile([64, SQ], F32, tag="t")
                m4 = tmppool.tile([64, SQ], F32, tag="t")
                eng.tensor_mul(m1[:], lo, cs_lo)
                eng.tensor_mul(m2[:], hi, sn_hi)
                eng.tensor_mul(m3[:], lo, sn_lo)
                eng.tensor_mul(m4[:], hi, cs_hi)
                eng.tensor_sub(hi, m1[:], m2[:])   # rotated low half
                eng.tensor_add(lo, m3[:], m4[:])   # rotated high half
            # transpose this chunk's V tiles: [hd, s] -> [s, hd]
            for tt in range(4):
                tp = ps8.tile([128, 128], F32R, tag="ps", name=f"vtp{s}_{tt}")
                nc.tensor.transpose(
                    tp[:], qkv[s][HPC + 1][:, tt * 128 : (tt + 1) * 128], ident[:]
                )
                nc.scalar.copy(vt_s[s][:, tt * 128 : (tt + 1) * 128], tp[:])

        # ---- fused QKV projection, two s-super-blocks (weights stream twice,
        # 2 x 12.6 MB).  Each super-block covers two s-chunks through all
        # d-groups; after its last group each s-chunk is folded, roped, and
        # its attention chunk emitted, so attention overlaps the next
        # super-block's (DMA-fed) projection. ----
        def proj_group(g, s_list):
            wchs = []
            for di in range(GRP):
                dd = g * GRP + di
                wch = wpool.tile([128, NT * 128], F32R, tag="w", name=f"w{dd}")
                nc.sync.dma_start(wch[:], wcat.ap()[dd * 128 : (dd + 1) * 128, :])
                wchs.append(wch)
            for s in s_list:
                ps = [
                    ps8.tile([128, SQ], F32, tag="ps", name=f"pp{s}_{g}_{i}")
                    for i in range(NT)
                ]
                last = g == NGRP - 1
                for di in range(GRP):
                    dd = g * GRP + di
                    xch = xpool.tile([128, SQ], F32R, tag="x")
                    nc.sync.dma_start(
                        xch[:],
                        xt.ap()[dd * 128 : (dd + 1) * 128, s * SQ : (s + 1) * SQ],
                    )
                    for nt in range(NT):
                        nc.tensor.matmul(
                            ps[nt][:],
                            wchs[di][:, nt * 128 : (nt + 1) * 128],
                            xch[:],
                            start=(di == 0),
                            stop=(di == GRP - 1 and not last),
                        )
                if last:
                    # fold the accumulated SBUF partial into PSUM on the PE
                    # itself (identity matmul), keeping DVE free for RoPE;
                    # ACT then writes the final value back to SBUF.
                    for nt in range(NT):
                        nc.tensor.matmul(
                            ps[nt][:],
                            ident[:],
                            qkv[s][nt][:],
                            start=False,
                            stop=True,
                        )
                    for nt in range(NT):
                        nc.scalar.copy(qkv[s][nt][:], ps[nt][:])
                    if s == 0:
                        rope_and_vt(0)
                    else:
                        # software-pipeline: attention chunk s-1 is fully
                        # finalized by now; emit it, then finalize s's rope.
                        attn_chunk(s - 1)
                        rope_and_vt(s)
                else:
                    for nt in range(NT):
                        if g == 0:
                            nc.scalar.copy(qkv[s][nt][:], ps[nt][:])
                        else:
                            nc.vector.tensor_add(
                                qkv[s][nt][:], qkv[s][nt][:], ps[nt][:]
                            )

        def ktile(t):
            return qkv[t // 4][HPC][:, (t % 4) * 128 : (t % 4) * 128 + 128]

        def vtile(t):
            return vt_s[t // 4][:, (t % 4) * 128 : (t % 4) * 128 + 128]

        # ---- attention (flash, transposed scores, causal block skip) ----
        def attn_chunk(c):
            for h in range(HPC):
                qmv = qkv[c][h][:]
                av = ps8.tile([128, SQ], F32, tag="ps", name=f"av{h}_{c}")
                den = ps8.tile([128, SQ], F32, tag="ps", name=f"den{h}_{c}")
                ntiles = 4 * c + 4
                for t in range(ntiles):
                    sc = ps8.tile([128, SQ], F32, tag="ps", name=f"sc{h}_{c}_{t}")
                    nc.tensor.matmul(sc[:], ktile(t), qmv, start=True, stop=True)
                    j = t - 4 * c
                    if j >= 0:
                        nc.vector.tensor_add(
                            sc[:], sc[:], dmask[:, j * SQ : (j + 1) * SQ]
                        )
                    pt = ptpool.tile([128, SQ], F32R, tag="pt")
                    nc.scalar.activation(pt[:], sc[:], Exp)
                    nc.tensor.matmul(
                        av[:],
                        vtile(t),
                        pt[:],
                        start=(t == 0),
                        stop=(t == ntiles - 1),
                    )
                    nc.tensor.matmul(
                        den[:],
                        ones_t[:],
                        pt[:],
                        start=(t == 0),
                        stop=(t == ntiles - 1),
                    )
                rec = recpool.tile([128, SQ], F32, tag="rec")
                nc.vector.reciprocal(rec[:], den[:])
                nc.vector.tensor_mul(
                    attout[c][:, h * SQ : (h + 1) * SQ], av[:], rec[:]
                )

        proj_group(0, list(range(NSQ)))
        load_consts()
        for g in range(1, NGRP):
            proj_group(g, list(range(NSQ)))
        attn_chunk(NSQ - 1)

        # ---- output projection (partial sums; host reduces across cores) ----
        for j in range(D // SQ):
            woch = wostream.tile([128, HPC * SQ], F32R, tag="woch", name=f"woch{j}")
            for hh in range(HPC):
                nc.sync.dma_start(
                    woch[:, hh * SQ : (hh + 1) * SQ],
                    wor.ap()[:, hh * D + j * SQ : hh * D + (j + 1) * SQ],
                )
            for m in range(NKT):
                ao = attout[m // 4]
                mo = (m % 4) * 128
                po = ps8.tile([128, SQ], F32, tag="ps", name=f"po{m}_{j}")
                for hh in range(HPC):
                    nc.tensor.matmul(
                        po[:],
                        ao[:, hh * SQ + mo : hh * SQ + mo + 128],
                        woch[:, hh * SQ : (hh + 1) * SQ],
                        start=(hh == 0),
                        stop=(hh == HPC - 1),
                    )
                st = stpool.tile([128, SQ], F32, tag="st")
                nc.scalar.copy(st[:], po[:])
                nc.sync.dma_start(
                    out.ap()[m * 128 : (m + 1) * 128, j * SQ : (j + 1) * SQ], st[:]
                )


def _host_prep(x, wq, wk, wv, wo, freqs_cos, freqs_sin):
    """Build the 8 per-core input maps."""
    perm = np.concatenate([np.arange(0, HD, 2), np.arange(1, HD, 2)])
    xt = np.ascontiguousarray(x.reshape(S, D).T)
    cosT = np.ascontiguousarray(freqs_cos.T.astype(np.float32))
    sinT = np.ascontiguousarray(freqs_sin.T.astype(np.float32))
    # diagonal-block causal masks: block j of a 512-query chunk vs its 128-key tile
    kk = np.arange(128)[:, None]
    qq = np.arange(SQ)[None, :]
    diagm = np.concatenate(
        [
            np.where(128 * j + kk <= qq, 0.0, -1e9).astype(np.float32)
            for j in range(4)
        ],
        axis=1,
    )
    ones = np.ones((128, 128), np.float32)
    ident = np.eye(128, dtype=np.float32)
    scale = 1.0 / math.sqrt(HD)

    in_maps = []
    for c in range(NCORES):
        wq_c = (
            wq[:, (HPC * c) * HD : (HPC * c + HPC) * HD]
            .reshape(D, HPC, HD)[:, :, perm]
            .reshape(D, HPC * HD)
            * scale
        )
        wk_c = wk[:, c * HD : (c + 1) * HD][:, perm]
        wv_c = wv[:, c * HD : (c + 1) * HD]
        wcat = np.ascontiguousarray(
            np.concatenate([wq_c, wk_c, wv_c], axis=1), dtype=np.float32
        )
        # wo rows for this core's heads: [HPC*HD, D] -> [128, HPC*D]
        wo_c = wo[(HPC * c) * HD : (HPC * c + HPC) * HD, :].reshape(HPC, 128, D)
        wor = np.ascontiguousarray(wo_c.transpose(1, 0, 2).reshape(128, HPC * D))
        in_maps.append(
            {
                "xt": xt,
                "wcat": wcat,
                "wor": wor,
                "cost": cosT,
                "sint": sinT,
                "diagm": diagm,
                "onesd": ones,
                "identd": ident,
            }
        )
    return in_maps


def _numpy_fallback(x, wq, wk, wv, wo, freqs_cos, freqs_sin, mask):
    """Exact reference math in numpy (used only for non-causal masks)."""
    bsz = x.shape[0]
    n_rep = H // H_KV
    xq = (x.reshape(-1, D) @ wq).reshape(bsz, S, H, HD)
    xk = (x.reshape(-1, D) @ wk).reshape(bsz, S, H_KV, HD)
    xv = (x.reshape(-1, D) @ wv).reshape(bsz, S, H_KV, HD)

    def rope(t):
        t0, t1 = t[..., 0::2], t[..., 1::2]
        c = freqs_cos[None, :, None, :]
        s = freqs_sin[None, :, None, :]
        o0 = t0 * c - t1 * s
        o1 = t0 * s + t1 * c
        return np.stack([o0, o1], axis=-1).reshape(t.shape)

    xq, xk = rope(xq), rope(xk)
    keys = np.repeat(xk, n_rep, axis=2)
    values = np.repeat(xv, n_rep, axis=2)
    scores = np.einsum("bqhd,bkhd->bhqk", xq, keys) / math.sqrt(HD)
    scores = scores + mask[:, :, -S:, -S:]
    scores = scores - scores.max(axis=-1, keepdims=True)
    e = np.exp(scores)
    attn = e / e.sum(axis=-1, keepdims=True)
    o = np.einsum("bhqk,bkhd->bqhd", attn, values).reshape(bsz, S, H * HD)
    return (o @ wo).astype(np.float32)


def kernel(**inputs):
    x = np.asarray(inputs["x"], dtype=np.float32)
    wq = np.asarray(inputs["wq"], dtype=np.float32)
    wk = np.asarray(inputs["wk"], dtype=np.float32)
    wv = np.asarray(inputs["wv"], dtype=np.float32)
    wo = np.asarray(inputs["wo"], dtype=np.float32)
    fc = np.asarray(inputs["freqs_cos"], dtype=np.float32)
    fs = np.asarray(inputs["freqs_sin"], dtype=np.float32)
    mask = np.asarray(inputs["mask"], dtype=np.float32)

    causal = np.triu(np.full((S, S), -1e9, dtype=np.float32), k=1)[None, None]
    if x.shape != (1, S, D) or not np.array_equal(mask, causal):
        return _numpy_fallback(x, wq, wk, wv, wo, fc, fs, mask)

    if "nc" not in _NC_CACHE:
        _NC_CACHE["nc"] = _build_nc()
    nc = _NC_CACHE["nc"]
    in_maps = _host_prep(x[0], wq, wk, wv, wo, fc, fs)
    _log("launching on 8 cores (compile on first call + transfers)")
    res = run_bass_kernel_spmd(nc, in_maps, core_ids=list(range(NCORES)))
    _log("run complete")
    full = np.zeros((S, D), np.float32)
    for r in res.results:
        full += r["out"]
    return full.reshape(1, S, D)
